# revision 44
# baseline (speedup 1.0000x reference)
"""Valid 3x3x3 conv3d: x[2,32,64,64,64] (*) W[64,32,3,3,3] -> y[2,64,62,62,62].

Sharding: D axis split across 8 cores (8 output planes each, 2-plane input halo,
sliced host-side). Batch = 2 independent streams per core.

Per-core compute: conv as 27 shifted bf16 matmuls reduced to 6 per 8-row
block (x and W ship as bf16: same 1 cyc/row as fp32r, half the input DMA,
~0.3% quantization error vs the 2e-2 gate):
  - K = 96: in_c(32) x kz(3); plane d lives at partition group (d mod 3).
    Weight column layout is rotated per (output plane mod 3). Host ships x
    dz-major ([NB, PD+2, IC, 64*64]) so a 96-partition DMA spans 3 planes.
  - Per block: 3 pair matmuls (ky x {kx0|kx1}, M=128) + 3 singles
    (ky x kx2, M=64), accumulating into half of a 2-bank psum tile shared by
    a block PAIR; psum pool is 4 pairs deep (8 banks) so the combine never
    back-pressures the PE. The psum uses a PACKED-63 grid (strided rhs APs
    skip x column 63 of each row), saving 1.5% of matmul columns.
  - Combine per block: ACT copies the +1-shifted kx1 half to SBUF bf16, DVE
    adds. One output DMA per (stream, plane); host widens bf16 -> fp32.
  - The halo plane (dz=9) loads into the spare partition group 3 (96:128),
    so its DMA has no write-after-read hazard against plane PD-2 compute;
    the last plane contracts K=128 with zero weights on rows 0:32 (weight
    rotation slot 3) and both k=7 planes interleave at pair granularity
    with split output DMAs to shorten the drain tail.
  - PE p-state: two small warm-up matmuls on the first-arriving x chunk
    start the 2.4GHz clock ramp before real compute (~3.4us in).
"""
import sys
sys.path.insert(0, '/opt/trn_rl_repo')
import numpy as np

IN_C, OUT_C = 32, 64
SH = SW = 64
OD = 62
PD = 8          # output planes per core per batch
HALO = 2
NB = 2          # batches/streams
BLOCKS = [(h0, min(8, OD - h0)) for h0 in range(0, OD, 8)]  # 7x8 + 1x6
PAIRS = [BLOCKS[i:i + 2] for i in range(0, len(BLOCKS), 2)]  # 4 pairs

_cache = {}


def _npk(nh):
    return nh * 63              # packed-63 psum window per block


def _build():
    import concourse.bacc as bacc
    import concourse.mybir as mybir
    from concourse import tile
    dt = mybir.dt

    nc = bacc.Bacc(trn_type="TRN2")
    x_d = nc.declare_dram_parameter("x", [NB, PD + HALO, IN_C, SH * SW],
                                    dt.float32, isOutput=False)
    w_d = nc.declare_dram_parameter("w", [96, 3, 5, 128], dt.float32,
                                    isOutput=False)
    y_d = nc.declare_dram_parameter("y", [NB, OUT_C, PD, OD, OD],
                                    dt.bfloat16, isOutput=True)

    with tile.TileContext(nc) as tc:
        with tc.tile_pool(name="xb", bufs=1) as xb_pool, \
             tc.tile_pool(name="wb", bufs=1) as wb_pool, \
             tc.tile_pool(name="ps", bufs=4, space="PSUM") as ps_pool, \
             tc.tile_pool(name="cb", bufs=4) as cb_pool, \
             tc.tile_pool(name="pb", bufs=2) as pb_pool:

            wbuf = wb_pool.tile([96, 3, 5, 128], dt.float32r)
            xbuf = xb_pool.tile([128, NB, SH * SW + 4], dt.float32r)

            # stream-0 planes 0..2 land as 96-partition DMAs in column
            # chunks (dz-major host layout makes the partition map affine);
            # chunk 0 first so the warm-up and first blocks start early
            cw = SH * SW // 4
            nc.sync.dma_start(out=xbuf[0:96, 0, 0:cw],
                              in_=x_d[0, 0:3, :, 0:cw].rearrange(
                                  "d i c -> (d i) c").bitcast(dt.float32r))
            # warm-up: start the PE p-state ramp on the first chunk
            wt = ps_pool.tile([128, 2, 512], dt.float32, tag="b")
            for _ in range(2):
                nc.tensor.matmul(wt[:, 0, 0:256], xbuf[0:96, 0, 0:128],
                                 xbuf[0:96, 0, 0:256], start=True, stop=True)
            # weights rot 0 (k=0 uses it), then the rest of the planes
            nc.sync.dma_start(out=wbuf[:, 0:1, :, :],
                              in_=w_d[:, 0:1, :, :].bitcast(dt.float32r))
            for ci in range(1, 4):
                nc.sync.dma_start(
                    out=xbuf[0:96, 0, ci * cw:(ci + 1) * cw],
                    in_=x_d[0, 0:3, :, ci * cw:(ci + 1) * cw].rearrange(
                        "d i c -> (d i) c").bitcast(dt.float32r))
            nc.sync.dma_start(out=wbuf[:, 1:3, :, :],
                              in_=w_d[:, 1:3, :, :].bitcast(dt.float32r))
            nc.sync.dma_start(out=xbuf[0:96, 1, 0:SH * SW],
                              in_=x_d[1, 0:3, :, :].rearrange(
                                  "d i c -> (d i) c").bitcast(dt.float32r))

            def load_plane(s, dz):
                g = dz % 3
                nc.sync.dma_start(
                    out=xbuf[g * 32:(g + 1) * 32, s, 0:SH * SW],
                    in_=x_d[s, dz, :, :].bitcast(dt.float32r))

            def emit_combine(pt, pbuf, c, ci, h0, nh):
                """packed-63 psum half ci -> pbuf rows h0:h0+nh."""
                pvb = pt[:, ci, 0:_npk(nh)].rearrange(
                    "p (h w) -> p h w", w=63)
                p0 = pvb[0:64, 0:nh, 0:62]
                p1 = pvb[64:128, 0:nh, 1:63]
                cc = c[:, ci, 0:nh, :]
                o = pbuf[:, h0:h0 + nh, :]
                nc.scalar.copy(cc, p1)
                nc.vector.tensor_add(o, p0, cc)

            def compute_plane(s, k, fine_tail=False):
                r = k % 3
                pbuf = pb_pool.tile([64, OD, OD], dt.bfloat16)
                for gi, group in enumerate(PAIRS):
                    pt = ps_pool.tile([128, 2, 512], dt.float32, tag="b")
                    for bi, (h0, nh) in enumerate(group):
                        base = h0 * 64

                        def rhs(off):
                            return xbuf[0:hi, s,
                                        base + off:base + off + nh * 64
                                        ].rearrange("p (h w) -> p h w",
                                                    w=64)[:, :, 0:63]
                        for t in range(3):  # pairs (kx0|kx1), M=128
                            nc.tensor.matmul(
                                pt[:, bi, 0:_npk(nh)],
                                wbuf[0:hi, r, t, 0:128],
                                rhs(64 * t), start=(t == 0), stop=False)
                        for t in range(3):  # singles kx2, M=64
                            wsl = (wbuf[0:hi, r, 3, t * 64:t * 64 + 64]
                                   if t < 2 else wbuf[0:hi, r, 4, 0:64])
                            nc.tensor.matmul(
                                pt[0:64, bi, 0:_npk(nh)],
                                wsl,
                                rhs(64 * t + 2), start=False, stop=(t == 2))

                    # combine: out = P0 + shift1(P1), bf16
                    (h0a, nh0), (h0b, nh1) = group
                    if only_bi is None:
                        c = cb_pool.tile([64, 2, 8, 62], dt.bfloat16)
                        emit_combine(pt, pbuf, c, 0, h0a, nh0)
                        emit_combine(pt, pbuf, c, 1, h0b, nh1)
                    else:
                        c = cb_pool.tile([64, 2, 8, 62], dt.bfloat16)
                        hb, nb_ = (h0a, nh0) if only_bi == 0 else (h0b, nh1)
                        emit_combine(pt, pbuf, c, only_bi, hb, nb_)
                    if interleave_s1 and gi == 3:
                        hb, nb_ = (h0a, nh0) if only_bi == 0 else (h0b, nh1)
                        nc.sync.dma_start(
                            out=y_d[s, :, k, hb:hb + nb_, :],
                            in_=pbuf[:, hb:hb + nb_, :])
                    elif interleave_s1 and only_bi == 1:
                        nc.sync.dma_start(
                            out=y_d[s, :, k, h0a:h0b + nh1, :],
                            in_=pbuf[:, h0a:h0b + nh1, :])
                if not interleave_s1:
                    for s, pbuf in streams:
                        nc.sync.dma_start(out=y_d[s, :, k, :, :],
                                          in_=pbuf[:, :, :])
                if prefetch is not None:
                    load_plane(s, prefetch)

            for k in range(PD):
                for s in range(NB):
                    compute_plane(s, k, fine_tail=(k == PD - 1 and s == NB - 1))
                    if k + 3 < PD + HALO:
                        load_plane(s, k + 3)

    nc.compile()
    return nc


def _weights_rot(Wf):
    """[96, 3(rot), 5(pass), 128] with kz=(g-r)%3 per partition group g.

    pass t in 0..2: cols 0:64 = (ky=t, kx=0).T, cols 64:128 = (ky=t, kx=1).T
    pass 3: singles (ky=0,kx=2).T at cols 0:64, (ky=1,kx=2).T at 64:128
    pass 4: single (ky=2,kx=2).T at cols 0:64
    """
    Wr = np.zeros((128, 4, 5, 128), np.float32)
    for r in range(3):
        for g in range(3):
            kz = (g - r) % 3
            sl = slice(g * 32, (g + 1) * 32)
            blk = Wf[:, :, kz, :, :]  # [oc, ic, ky, kx]
            for t in range(3):
                Wr[sl, r, t, 0:64] = blk[:, :, t, 0].T
                Wr[sl, r, t, 64:128] = blk[:, :, t, 1].T
            Wr[sl, r, 3, 0:64] = blk[:, :, 0, 2].T
            Wr[sl, r, 3, 64:128] = blk[:, :, 1, 2].T
            Wr[sl, r, 4, 0:64] = blk[:, :, 2, 2].T
    Wr[32:128, 3] = Wr[0:96, 0]  # r=0 layout shifted to partitions 32:128
    return Wr


def kernel(x, W):
    from concourse.bass_utils import run_bass_kernel_spmd
    x = np.ascontiguousarray(np.asarray(x), np.float32)
    W = np.ascontiguousarray(np.asarray(W), np.float32)
    if "nc" not in _cache:
        _cache["nc"] = _build()
    nc = _cache["nc"]

    import ml_dtypes
    bf16 = ml_dtypes.bfloat16
    xp = np.zeros((NB, IN_C, 8 * PD + HALO, SH, SW), np.float32)
    xp[:, :, :64] = x
    Wr = _weights_rot(W).astype(bf16)
    # dz-major per-core slabs: [NB, PD+2, IC, SH*SW], bf16
    xpf = xp.reshape(NB, IN_C, 8 * PD + HALO, SH * SW).transpose(
        0, 2, 1, 3).astype(bf16)
    in_maps = [{"x": np.ascontiguousarray(xpf[:, c * PD:c * PD + PD + HALO]),
                "w": Wr} for c in range(8)]
    res = run_bass_kernel_spmd(nc, in_maps, core_ids=list(range(8)))

    out = np.empty((NB, OUT_C, OD, OD, OD), np.float32)
    for c in range(8):
        lo = c * PD
        n = min(PD, OD - lo)
        if n > 0:
            out[:, :, lo:lo + n] = np.asarray(
                res.results[c]["y"][:, :, :n]).astype(np.float32)
    return out
